# revision 1
# baseline (speedup 1.0000x reference)
"""SAGAN-style self-attention block on 8 Trainium2 NeuronCores.

Reference computation (per batch element b, C=128, H=W=64, N=4096):
    theta = W_theta @ x_b                       [16, 4096]
    phi   = maxpool2x2(W_phi @ x_b)             [16, 1024]
    g     = maxpool2x2(W_g @ x_b)               [64, 1024]
    S     = theta^T phi                         [4096, 1024]
    beta  = softmax(S, axis=-1)
    o     = g @ beta^T                          [64, 4096]
    out   = gamma * (W_o @ o) + x_b             [128, 4096]

Sharding: data-parallel over batch; core b gets batch element b; weights
replicated; no collectives.

Device dataflow (computes S^T = phi^T theta so softmax's reduction axis
lands on the PE contraction axis; row-sums come for free from a ones
column folded into the g^T stationary operand):
    theta_rep [112, 4096]  theta replicated at partition groups 0/32/64/96
                           (via a replicated conv weight) so four K=16
                           matmuls can pack into the PE's four row-groups
    phi_rep   [112, 1024]  same replication
    S^T chunk [128m, 512n] = phi_chunk^T @ theta_chunk   x4 concurrent
    E^T = exp(S^T)         (no max subtraction: |S| <= ~12, exp safe)
    po [128, 512] = sum_m gTa_m^T @ E^T_m,  gTa = [1 | 0*63 | g^T]
        -> row 0 = s_n (softmax denominator), rows 64..127 = unnorm o
    o2 = W_o @ o           (operands on partitions 64..127, tile_position
                           (64,0) since K=64 requires 64-alignment)
    out = (o2 * gamma) * (1/s broadcast via ones-matmul) + x

Matmul operands use the FP32R format (fp32 with mantissa rounded to 11
bits; full-rate PE streaming vs 1/4-rate fp32). Host inputs are
pre-rounded; on-device producers write float32r APs so the engines round
on the write port (walrus checkMatmultFP32r requires rounded producers).
"""

import os
import numpy as np

MM_MODE = os.environ.get("K_MM_MODE", "f32r")  # f32r | f32
ET_BF16 = os.environ.get("K_ET_BF16", "1") == "1"  # bf16 attention weights
N_CORES = 8
C = 128
N = 4096       # H*W
M = 1024       # N/4
NCH = 8        # n-chunks
CHUNK = 512


def _round_fp32r(a: np.ndarray) -> np.ndarray:
    """Round fp32 to the FP32R grid (11-bit mantissa, round-half-even)."""
    u = np.ascontiguousarray(a, dtype=np.float32).view(np.uint32)
    lsb = (u >> np.uint32(12)) & np.uint32(1)
    r = (u + np.uint32(0x7FF) + lsb) & np.uint32(0xFFFFF000)
    return r.view(np.float32)


def _build(gamma: float, reps: int = 1):
    from contextlib import nullcontext
    import concourse.bass as bass
    import concourse.tile as tile
    from concourse import bacc, mybir

    f32 = mybir.dt.float32
    fmm = mybir.dt.float32r if MM_MODE == "f32r" else f32
    fet = mybir.dt.bfloat16 if ET_BF16 else fmm
    ts = bass.ts
    ALU = mybir.AluOpType
    ACTF = mybir.ActivationFunctionType

    nc = bacc.Bacc(
        "TRN2", target_bir_lowering=False, debug=False, enable_asserts=False,
        num_devices=N_CORES,
    )
    x_d = nc.dram_tensor("x", [C, N], f32, kind="ExternalInput")
    xr_d = nc.dram_tensor("xr", [C, N], fmm, kind="ExternalInput")
    # all matmul weights packed in one DMA: cols 0:112 wt_rep, 112:224 wp_rep,
    # 224:288 wg_t, 288:416 wo_t (rows 64:128)
    wcat_d = nc.dram_tensor("wcat", [128, 416], fmm, kind="ExternalInput")
    id_d = nc.dram_tensor("ident", [64, 64], f32, kind="ExternalInput")
    out_d = nc.dram_tensor("out", [C, N], f32, kind="ExternalOutput")

    with tile.TileContext(nc) as tc:
        with (
            tc.tile_pool(name="persist", bufs=1) as persist,
            tc.tile_pool(name="et", bufs=8) as etp,
            tc.tile_pool(name="work", bufs=3) as work,
            tc.tile_pool(name="outp", bufs=3) as outpool,
            tc.tile_pool(name="pspair", bufs=2, space="PSUM") as pspair,
            tc.tile_pool(name="psacc", bufs=1, space="PSUM") as psacc,
            tc.tile_pool(name="pssm", bufs=3, space="PSUM") as pssm,
        ):
          loop_cm = (
              tc.For_i(
                  0, reps, 1,
                  hint_engines=(
                      mybir.EngineType.PE,
                      mybir.EngineType.DVE,
                      mybir.EngineType.Activation,
                      mybir.EngineType.SP,
                      mybir.EngineType.Pool,
                  ),
              )
              if reps > 1
              else nullcontext()
          )
          with loop_cm:
            # ---- constants / inputs -------------------------------------
            # weights first: HWDGE drains FIFO per ring, and the first conv
            # matmul needs wcat — issuing it after the 8MB of x would stall
            # the whole head ~20us.
            wcat = persist.tile([128, 416], fmm, name="wcat")
            nc.sync.dma_start(wcat, wcat_d[:, :])
            id_sb = persist.tile([64, 64], f32, name="id_sb")
            nc.sync.dma_start(id_sb, id_d[:, :])
            X32 = persist.tile([C, N], f32, name="X32")
            Xr = persist.tile([C, N], fmm, name="Xr")
            for k in range(4):
                nc.sync.dma_start(Xr[:, ts(k, 1024)], xr_d[:, ts(k, 1024)])
            for k in range(2):
                nc.sync.dma_start(X32[:, ts(k, 2048)], x_d[:, ts(k, 2048)])
            wt_sb = wcat[:, 0:112]
            wp_sb = wcat[:, 112:224]
            wg_sb = wcat[:, 224:288]
            wo_sb = wcat[:, 288:416]
            ones_f32 = persist.tile([1, 128], f32, name="ones_f32")
            nc.vector.memset(ones_f32, 1.0)
            ones_sb = persist.tile([1, 128], fmm, name="ones_sb")
            nc.vector.tensor_copy(ones_sb, ones_f32)

            # [1 | 0*63] template: fills the first 64 columns of each gTa
            # block (ones column for the row-sum, zero padding to align g^T
            # at partition 64).
            onecol = persist.tile([128, 64], f32, name="onecol")
            nc.vector.memset(onecol, 0.0)
            nc.vector.memset(onecol[:, 0:1], 1.0)

            theta_rep = persist.tile([112, N], fmm, name="theta_rep")
            phi_rep = persist.tile([112, M], fmm, name="phi_rep")
            g_sb = persist.tile([64, M], f32, name="g_sb")
            gTa = persist.tile([128, 8 * 128], fet, name="gTa")
            for mi in range(8):
                nc.vector.tensor_copy(gTa[:, mi * 128 : mi * 128 + 64], onecol)

            # ---- convs + pooling ---------------------------------------
            for ci in range(NCH):
                xc = Xr[:, ts(ci, CHUNK)]
                pph = pspair.tile([112, CHUNK], f32, name="pph", tag="pair")
                nc.tensor.matmul(pph, wp_sb, xc, start=True, stop=True)
                nc.vector.tensor_reduce(
                    out=phi_rep[:, ts(ci, 128)].rearrange("p (i j) -> p i j", i=4, j=32),
                    in_=pph.rearrange(
                        "p (i di j dj) -> p i j di dj", i=4, di=2, j=32, dj=2
                    ),
                    axis=mybir.AxisListType.XY,
                    op=ALU.max,
                )

                pg = pspair.tile([64, CHUNK], f32, name="pg", tag="pair")
                nc.tensor.matmul(pg, wg_sb, xc, start=True, stop=True)
                nc.vector.tensor_reduce(
                    out=g_sb[:, ts(ci, 128)].rearrange("p (i j) -> p i j", i=4, j=32),
                    in_=pg.rearrange(
                        "p (i di j dj) -> p i j di dj", i=4, di=2, j=32, dj=2
                    ),
                    axis=mybir.AxisListType.XY,
                    op=ALU.max,
                )

                pth = pspair.tile([112, CHUNK], f32, name="pth", tag="pair")
                nc.tensor.matmul(pth, wt_sb, xc, start=True, stop=True)
                nc.vector.tensor_copy(theta_rep[:, ts(ci, CHUNK)], pth)

            # ---- g^T via PE transpose ----------------------------------
            for mi in range(8):
                ptr = pssm.tile([128, 64], f32, name="ptr", tag="small")
                nc.tensor.transpose(ptr, g_sb[:, ts(mi, 128)], id_sb)
                nc.vector.tensor_copy(gTa[:, mi * 128 + 64 : mi * 128 + 128], ptr)

            # ---- attention over n-chunks -------------------------------
            for ci in range(NCH):
                ets = []
                for q in range(2):
                    pair_a = pspair.tile([128, 1024], f32, name="pair_a", tag="pair")
                    pair_b = pspair.tile([128, 1024], f32, name="pair_b", tag="pair")
                    for j in range(4):
                        mi = 4 * q + j
                        dst = (pair_a if j < 2 else pair_b)[
                            :, (j % 2) * CHUNK : (j % 2) * CHUNK + CHUNK
                        ]
                        nc.tensor.matmul(
                            dst,
                            phi_rep[32 * j : 32 * j + 16, ts(mi, 128)],
                            theta_rep[32 * j : 32 * j + 16, ts(ci, CHUNK)],
                            start=True,
                            stop=True,
                            tile_position=(32 * j, 0),
                        )
                    for pair in (pair_a, pair_b):
                        et = etp.tile([128, 1024], fet, name="et", tag="et")
                        nc.scalar.activation(et, pair, ACTF.Exp)
                        ets.append(et)

                po = psacc.tile([128, CHUNK], f32, name="po", tag="acc")
                for mi in range(8):
                    rhs = ets[mi // 2][:, (mi % 2) * CHUNK : (mi % 2) * CHUNK + CHUNK]
                    nc.tensor.matmul(
                        po,
                        gTa[:, mi * 128 : (mi + 1) * 128],
                        rhs,
                        start=(mi == 0),
                        stop=(mi == 7),
                    )

                o_sb = work.tile([128, CHUNK], fmm, name="o_sb", tag="osb")
                nc.vector.tensor_copy(o_sb[64:128, :], po[64:128, :])
                s_sb = work.tile([1, CHUNK], fmm, name="s_sb", tag="ssb")
                nc.vector.tensor_copy(s_sb, po[0:1, :])

                pbc = pssm.tile([128, CHUNK], f32, name="pbc", tag="small")
                nc.tensor.matmul(pbc, ones_sb, s_sb, start=True, stop=True)
                rbc = work.tile([128, CHUNK], f32, name="rbc", tag="rbc")
                nc.vector.reciprocal_approx_fast(rbc, pbc)

                po2 = pssm.tile([128, CHUNK], f32, name="po2", tag="small")
                nc.tensor.matmul(
                    po2,
                    wo_sb[64:128, :],
                    o_sb[64:128, :],
                    start=True,
                    stop=True,
                    tile_position=(64, 0),
                )

                t1 = work.tile([128, CHUNK], f32, name="t1", tag="t1")
                nc.vector.scalar_tensor_tensor(
                    t1, in0=po2, scalar=float(gamma), in1=rbc,
                    op0=ALU.mult, op1=ALU.mult,
                )
                if ci % 2 == 0:
                    outp = outpool.tile([128, 2 * CHUNK], f32, name="outp", tag="out")
                half = outp[:, (ci % 2) * CHUNK : (ci % 2) * CHUNK + CHUNK]
                nc.gpsimd.tensor_add(half, t1, X32[:, ts(ci, CHUNK)])
                if ci % 2 == 1:
                    nc.sync.dma_start(out_d[:, bass.ds((ci - 1) * CHUNK, 2 * CHUNK)], outp)

    nc.compile()
    return nc


def _host_prep(x, W_theta, W_phi, W_g, W_o):
    x = np.ascontiguousarray(np.asarray(x, dtype=np.float32))
    B = x.shape[0]
    rnd = _round_fp32r if MM_MODE == "f32r" else (lambda a: np.asarray(a, np.float32))
    wcat = np.zeros((128, 416), dtype=np.float32)
    for j in range(4):
        wcat[:, 32 * j : 32 * j + 16] = np.asarray(W_theta, np.float32).T
        wcat[:, 112 + 32 * j : 112 + 32 * j + 16] = np.asarray(W_phi, np.float32).T
    wcat[:, 224:288] = np.asarray(W_g, np.float32).T
    wcat[64:128, 288:416] = np.asarray(W_o, np.float32).T
    wcat = rnd(wcat)
    ident = np.eye(64, dtype=np.float32)
    in_maps = []
    for b in range(B):
        xb = np.ascontiguousarray(x[b].reshape(C, N))
        in_maps.append(
            {
                "x": xb,
                "xr": rnd(xb),
                "wcat": wcat,
                "ident": ident,
            }
        )
    return in_maps


def run(x, W_theta, W_phi, W_g, W_o, gamma, trace=False, **trace_kwargs):
    from concourse.bass_utils import run_bass_kernel_spmd

    nc = _build(float(np.asarray(gamma)))
    in_maps = _host_prep(x, W_theta, W_phi, W_g, W_o)
    res = run_bass_kernel_spmd(
        nc, in_maps, core_ids=list(range(N_CORES)), trace=trace, **trace_kwargs
    )
    outs = [res.results[b]["out"].reshape(C, 64, 64) for b in range(N_CORES)]
    return np.stack(outs).astype(np.float32), res


def kernel(x, W_theta, W_phi, W_g, W_o, gamma):
    out, _ = run(x, W_theta, W_phi, W_g, W_o, gamma)
    return out



# revision 5
# speedup vs baseline: 1.4380x; 1.4380x over previous
"""SAGAN-style self-attention block on 8 Trainium2 NeuronCores (v2).

Reference computation (per batch element b, C=128, H=W=64, N=4096, M=1024):
    theta = W_theta @ x_b                       [16, 4096]
    phi   = maxpool2x2(W_phi @ x_b)             [16, 1024]
    g     = maxpool2x2(W_g @ x_b)               [64, 1024]
    S     = theta^T phi                         [4096, 1024]
    beta  = softmax(S, axis=-1)
    o     = g @ beta^T                          [64, 4096]
    out   = gamma * (W_o @ o) + x_b             [128, 4096]

Sharding: data-parallel over batch; core b gets batch element b; weights
replicated; no collectives.

v2 design notes (engine budgets from the TimelineSim cost model; op cost
scales with free-dim columns only, GPSIMD/Pool cannot touch PSUM):
  - bf16 on all matmul paths (1 cyc/row PE streaming like fp32r, half the
    DMA), f32 psum, bf16 residual (total rel err ~2e-3 vs 2e-2 gate).
    gamma folded into W_o on the host.
  - Act engine is the hard floor: exp = 32 x [128,1024] tiles ~32us and
    only the scalar engine has Exp, so Act does exp and nothing else.
  - conv A: stationary [128K, 80M] = [W_g^T | W_phi^T] -> psum rows 0:64
    g, 64:80 phi; ONE fused maxpool tensor_reduce drains both (DVE).
    conv B: stationary [128K, 16M] = W_theta^T with tile_position=(0,64)
    so theta lands on partition band 64 = phi's band: the K=16 S^T
    matmul needs stationary phi and moving theta on one 32-aligned band.
  - S^T chunk [128m, 1024n] = phi_mi^T theta (32 matmuls). exp -> et
    bf16. po[128,512] accumulates gTa_mi^T et_mi with the ones-column
    trick (row 0 = softmax denominator, rows 64:128 = unnorm o). The po
    matmuls trail one mi behind the S^T matmuls in the PE stream so the
    PE fills its exp-wait gaps; the two chunks per superchunk accumulate
    in parallel psum banks.
  - Epilogue per 512-chunk: s->bf16 copy (DVE), broadcast via ones
    matmul (PE), reciprocal_approx_fast (DVE), o*(1/s) fused into the
    psum->sbuf drain (DVE), po2 = (gamma W_o) @ o_norm (PE),
    out = po2 + x (DVE), DMA per 1024 cols.
  - PSUM exactly 8 banks: pst 2x[128,1024] (4) + psm 4x[*,512] (4)
    shared by convs / po accumulators / pbc / po2 in rotation.
"""

import numpy as np

N_CORES = 8
C = 128
N = 4096       # H*W
M = 1024       # N/4
NCH = 8        # 512-col chunks
CHUNK = 512


def _build(gamma: float, reps: int = 1):
    from contextlib import nullcontext
    import concourse.bass as bass
    import concourse.tile as tile
    from concourse import bacc, mybir

    f32 = mybir.dt.float32
    bf16 = mybir.dt.bfloat16
    ts = bass.ts
    ALU = mybir.AluOpType
    ACTF = mybir.ActivationFunctionType

    nc = bacc.Bacc(
        "TRN2", target_bir_lowering=False, debug=False, enable_asserts=False,
        num_devices=N_CORES,
    )
    xb_d = nc.dram_tensor("xb", [C, N], bf16, kind="ExternalInput")
    # cols 0:64 W_g^T, 64:80 W_phi^T, 80:96 W_theta^T,
    # 96:224 (gamma*W_o)^T on rows 64:128
    wcat_d = nc.dram_tensor("wcat", [128, 224], bf16, kind="ExternalInput")
    id_d = nc.dram_tensor("ident", [64, 64], bf16, kind="ExternalInput")
    out_d = nc.dram_tensor("out", [C, N], f32, kind="ExternalOutput")

    with tile.TileContext(nc) as tc:
        with (
            tc.tile_pool(name="persist", bufs=1) as persist,
            tc.tile_pool(name="et", bufs=10) as etp,
            tc.tile_pool(name="osb", bufs=3) as osbp,
            tc.tile_pool(name="rcb", bufs=3) as rcbp,
            tc.tile_pool(name="ssb", bufs=3) as ssbp,
            tc.tile_pool(name="outp", bufs=3) as outpool,
            tc.tile_pool(name="pst", bufs=2, space="PSUM") as pst,
            tc.tile_pool(name="psm", bufs=4, space="PSUM") as psm,
        ):
          loop_cm = (
              tc.For_i(
                  0, reps, 1,
                  hint_engines=(
                      mybir.EngineType.PE,
                      mybir.EngineType.DVE,
                      mybir.EngineType.Activation,
                      mybir.EngineType.SP,
                      mybir.EngineType.Pool,
                  ),
              )
              if reps > 1
              else nullcontext()
          )
          with loop_cm:
            # ---- loads (weights first: first conv needs wcat) ----------
            wcat = persist.tile([128, 224], bf16, name="wcat")
            nc.sync.dma_start(wcat, wcat_d[:, :])
            id_sb = persist.tile([64, 64], bf16, name="id_sb")
            nc.sync.dma_start(id_sb, id_d[:, :])
            xb = persist.tile([C, N], bf16, name="xb")
            for q in range(4):
                nc.sync.dma_start(xb[:, ts(q, 1024)], xb_d[:, ts(q, 1024)])

            wA = wcat[:, 0:80]          # [W_g^T | W_phi^T]
            wB = wcat[:, 80:96]         # W_theta^T
            wo = wcat[64:128, 96:224]   # (gamma*W_o)^T, K band 64

            ones_sb = persist.tile([1, 128], bf16, name="ones_sb")
            nc.vector.memset(ones_sb, 1.0)

            theta_sb = persist.tile([80, N], bf16, name="theta_sb")  # rows 64:80
            # rows 0:64 g, rows 64:80 phi
            phig_sb = persist.tile([80, M], bf16, name="phig_sb")
            gTa = persist.tile([128, 8 * 128], bf16, name="gTa")
            nc.gpsimd.memset(gTa, 0.0)
            for mi in range(8):
                nc.vector.memset(gTa[:, mi * 128 : mi * 128 + 1], 1.0)

            # ---- conv phase (pipelines into attention) -----------------
            for ci in range(NCH):
                xc = xb[:, ts(ci, CHUNK)]
                psA = psm.tile([80, CHUNK], f32, name="psA", tag="sm")
                nc.tensor.matmul(psA, wA, xc, start=True, stop=True,
                                 tile_position=(0, 0))
                psB = psm.tile([128, CHUNK], f32, name="psB", tag="sm")
                nc.tensor.matmul(psB[64:80, :], wB, xc, start=True, stop=True,
                                 tile_position=(0, 64))
                # fused g+phi maxpool (DVE) — phi is latency-critical
                nc.vector.tensor_reduce(
                    out=phig_sb[:, ts(ci, 128)].rearrange(
                        "p (i j) -> p i j", i=4, j=32),
                    in_=psA.rearrange(
                        "p (i di j dj) -> p i j di dj", i=4, di=2, j=32, dj=2),
                    axis=mybir.AxisListType.XY,
                    op=ALU.max,
                )
                # theta copy (DVE)
                nc.vector.tensor_copy(theta_sb[64:80, ts(ci, CHUNK)],
                                      psB[64:80, :])

            # ---- g^T into gTa (ones col already set) -------------------
            for mi in range(8):
                ptr = psm.tile([128, 64], bf16, name="ptr", tag="sm")
                nc.tensor.transpose(ptr, phig_sb[0:64, ts(mi, 128)], id_sb)
                nc.vector.tensor_copy(gTa[:, mi * 128 + 64 : mi * 128 + 128],
                                      ptr)

            # ---- attention over n-superchunks of 1024 ------------------
            for k in range(4):
                po_a = psm.tile([128, CHUNK], f32, name="po_a", tag="sm")
                po_b = psm.tile([128, CHUNK], f32, name="po_b", tag="sm")
                pos = (po_a, po_b)
                ets = []

                def po_step(mi):
                    for c01 in range(2):
                        nc.tensor.matmul(
                            pos[c01], gTa[:, ts(mi, 128)],
                            ets[mi][:, ts(c01, CHUNK)],
                            start=(mi == 0), stop=(mi == 7),
                            tile_position=(0, 0))

                for mi in range(8):
                    st = pst.tile([128, 1024], f32, name="st", tag="st")
                    for h in range(2):
                        nc.tensor.matmul(
                            st[:, ts(h, CHUNK)], phig_sb[64:80, ts(mi, 128)],
                            theta_sb[64:80, bass.ds(k * 1024 + h * CHUNK, CHUNK)],
                            start=True, stop=True, tile_position=(64, 0))
                    et = etp.tile([128, 1024], bf16, name="et", tag="et")
                    nc.scalar.activation(et, st, ACTF.Exp)
                    ets.append(et)
                    # po trails S^T by one mi: PE fills its exp-wait gap
                    if mi >= 1:
                        po_step(mi - 1)
                po_step(7)

                for c01 in range(2):
                    c = 2 * k + c01
                    po = pos[c01]
                    s_sb = ssbp.tile([1, CHUNK], bf16, name="s_sb", tag="s")
                    nc.vector.tensor_copy(s_sb, po[0:1, :])
                    pbc = psm.tile([128, CHUNK], f32, name="pbc", tag="sm")
                    nc.tensor.matmul(pbc, ones_sb, s_sb, start=True, stop=True,
                                     tile_position=(0, 0))
                    rbc = rcbp.tile([128, CHUNK], f32, name="rbc", tag="r")
                    # full-tile recip: the custom DVE op silently no-ops on a
                    # partition-offset slice; cost is column-based anyway
                    nc.vector.reciprocal_approx_fast(rbc, pbc)
                    o_sb = osbp.tile([128, CHUNK], bf16, name="o_sb", tag="o")
                    nc.vector.tensor_tensor(o_sb[64:128, :], po[64:128, :],
                                            rbc[64:128, :], op=ALU.mult)
                    po2 = psm.tile([128, CHUNK], f32, name="po2", tag="sm")
                    nc.tensor.matmul(po2, wo, o_sb[64:128, :],
                                     start=True, stop=True,
                                     tile_position=(64, 0))
                    if c01 == 0:
                        outp = outpool.tile([128, 2 * CHUNK], f32,
                                            name="outp", tag="out")
                    nc.vector.tensor_tensor(outp[:, ts(c01, CHUNK)], po2,
                                            xb[:, ts(c, CHUNK)], op=ALU.add)
                    if c01 == 1:
                        nc.sync.dma_start(
                            out_d[:, bass.ds((c - 1) * CHUNK, 2 * CHUNK)],
                            outp)

    nc.compile()
    return nc


def _host_prep(x, W_theta, W_phi, W_g, W_o, gamma=0.5):
    import ml_dtypes
    bf16 = ml_dtypes.bfloat16
    x = np.ascontiguousarray(np.asarray(x, dtype=np.float32))
    B = x.shape[0]
    wcat = np.zeros((128, 224), dtype=np.float32)
    wcat[:, 0:64] = np.asarray(W_g, np.float32).T
    wcat[:, 64:80] = np.asarray(W_phi, np.float32).T
    wcat[:, 80:96] = np.asarray(W_theta, np.float32).T
    wcat[64:128, 96:224] = (float(gamma) * np.asarray(W_o, np.float32)).T
    wcat = wcat.astype(bf16)
    ident = np.eye(64, dtype=np.float32).astype(bf16)
    in_maps = []
    for b in range(B):
        xb = np.ascontiguousarray(x[b].reshape(C, N)).astype(bf16)
        in_maps.append({"xb": xb, "wcat": wcat, "ident": ident})
    return in_maps


def run(x, W_theta, W_phi, W_g, W_o, gamma, trace=False, **trace_kwargs):
    from concourse.bass_utils import run_bass_kernel_spmd

    nc = _build(float(np.asarray(gamma)))
    in_maps = _host_prep(x, W_theta, W_phi, W_g, W_o, float(np.asarray(gamma)))
    res = run_bass_kernel_spmd(
        nc, in_maps, core_ids=list(range(N_CORES)), trace=trace, **trace_kwargs
    )
    outs = [res.results[b]["out"].reshape(C, 64, 64) for b in range(N_CORES)]
    return np.stack(outs).astype(np.float32), res


def kernel(x, W_theta, W_phi, W_g, W_o, gamma):
    out, _ = run(x, W_theta, W_phi, W_g, W_o, gamma)
    return out
